# revision 4
# baseline (speedup 1.0000x reference)
"""Trainium2 Bass kernel for a causal attention block with softmax over the
QUERY axis (axis=1), data-parallel over batch across 8 NeuronCores.

Reference semantics (per batch element):
    q = x @ Wq + bq ; k = x @ Wk + bk ; v = x @ Wv + bv        # [T, 512]
    logits[t, s] = q[t] . k[s]   masked to s <= t (causal)
    probs = softmax(logits / sqrt(512), axis=t)                # query axis!
    read[t] = sum_s probs[t, s] * v[s]
    out = concat(x, read)

Device kernel computes `read` for one batch element; the batch is sharded
1-per-core across 8 cores and the x-passthrough concat happens on host.

Key layout choice: we materialize the score matrix TRANSPOSED,
L'[s, t] = q[t].k[s], so the softmax reduction (over t) runs along the free
axis, and L' (post-exp) feeds the read matmul directly as the stationary
operand: read[t, v] = sum_s P'[s, t] * vn[s, v] with vn = v / r (the softmax
denominator r[s] is folded into v instead of normalizing the big matrix).
"""

import math
from contextlib import ExitStack

import numpy as np

T = 2048
C = 512  # input channels (contract dim of projections)
K = 512  # key dim (contract dim of logits)
V = 512
P = 128
NCORES = 8
SCALE = 1.0 / math.sqrt(float(K))
NEG = -1.0e9

NT = T // P  # 16 t-chunks (and s-chunks)
NCC = C // P  # 4 contract chunks for projections
NKC = K // P  # 4 contract chunks for logits
NJ = T // 512  # 4 512-wide t slices

# P' strip i covers t in [T0[i], T); strips stored back-to-back (bf16)
T0 = [512 * (i // 4) for i in range(NT)]
WIDTHS = [T - T0[i] for i in range(NT)]
OFFS = np.cumsum([0] + WIDTHS).tolist()  # OFFS[16] == 20480


def emit(tc, out_ap, x, wq, bq, wk, bk, wv, bv):
    import concourse.bass as bass
    import concourse.mybir as mybir
    from concourse.masks import make_identity

    nc = tc.nc
    f32 = mybir.dt.float32
    f32r = mybir.dt.float32r
    bf16 = mybir.dt.bfloat16
    Exp = mybir.ActivationFunctionType.Exp
    Ident = mybir.ActivationFunctionType.Identity
    AX = mybir.AxisListType.X
    ADD = mybir.AluOpType.add

    with ExitStack() as ctx:
        const = ctx.enter_context(tc.tile_pool(name="const", bufs=1))
        xnat = ctx.enter_context(tc.tile_pool(name="xnat", bufs=8))
        big = ctx.enter_context(tc.tile_pool(name="big", bufs=1))
        qk = ctx.enter_context(tc.tile_pool(name="qk", bufs=1))
        vpool = ctx.enter_context(tc.tile_pool(name="vpool", bufs=1))
        stat = ctx.enter_context(tc.tile_pool(name="stat", bufs=4))
        outp = ctx.enter_context(tc.tile_pool(name="outp", bufs=3))
        psum = ctx.enter_context(tc.tile_pool(name="psum", bufs=8, space="PSUM"))

        # ---- constants ----
        ident = const.tile([P, P], f32)
        make_identity(nc, ident)

        # additive causal masks for the diagonal 128x512 tiles; pattern m
        # (m = i mod 4): keep (0.0) where f >= p + 128*m else NEG
        masks = const.tile([P, 4 * 512], f32)
        nc.gpsimd.memset(masks, 0.0)
        for m in range(4):
            sl = masks[:, 512 * m : 512 * (m + 1)]
            nc.gpsimd.affine_select(
                out=sl,
                in_=sl,
                compare_op=mybir.AluOpType.is_ge,
                fill=NEG,
                base=-128 * m,
                channel_multiplier=-1,
                pattern=[[1, 512]],
            )

        ones1 = const.tile([1, P], f32r)
        onestg = xnat.tile([1, P], f32, tag="xnat")
        nc.vector.memset(onestg, 1.0)
        nc.gpsimd.tensor_copy(ones1, onestg)
        bvrow = const.tile([1, V], f32r)
        bvstg = xnat.tile([1, V], f32, tag="xnat")
        nc.sync.dma_start(bvstg, bv[None, :])
        nc.gpsimd.tensor_copy(bvrow, bvstg)

        bqk = const.tile([P, 8], f32)
        nc.sync.dma_start(bqk[:, 0:4], bq.rearrange("(c p) -> p c", p=P))
        nc.sync.dma_start(bqk[:, 4:8], bk.rearrange("(c p) -> p c", p=P))

        wq_sb = const.tile([P, NCC * 512], f32r)
        wk_sb = const.tile([P, NCC * 512], f32r)
        wv_sb = const.tile([P, NCC * 512], f32r)
        for c in range(NCC):
            for w_dram, w_sb in ((wq, wq_sb), (wk, wk_sb), (wv, wv_sb)):
                wstg = xnat.tile([P, 512], f32, tag="xnat", name=f"wstg{c}")
                nc.sync.dma_start(wstg, w_dram[P * c : P * (c + 1), :])
                nc.gpsimd.tensor_copy(w_sb[:, 512 * c : 512 * (c + 1)], wstg)

        # ---- load x and transpose to xT [c, t] (c on partitions) ----
        xT = big.tile([P, NCC * T], f32r, tag="big")  # strip c at [:, T*c : T*(c+1)]
        for tg in range(4):  # groups of 4 t-chunks
            xts = []
            for tb in range(4):
                t_idx = 4 * tg + tb
                xt = xnat.tile([P, C], f32, tag="xnat")
                nc.sync.dma_start(xt, x[P * t_idx : P * (t_idx + 1), :])
                xts.append(xt)
            for c in range(NCC):
                pt = psum.tile([P, 512], f32, tag="ps")
                for tb in range(4):
                    nc.tensor.matmul(
                        pt[:, P * tb : P * (tb + 1)],
                        xts[tb][:, P * c : P * (c + 1)],
                        ident,
                        is_transpose=True,
                        start=(tb == 0),
                        stop=(tb == 3),
                    )
                nc.vector.tensor_copy(xT[:, T * c + 512 * tg : T * c + 512 * (tg + 1)], pt)

        # ---- v projection (natural layout [s, v]) + bias via K=1 matmul ----
        vsb = vpool.tile([P, NT * 512], bf16)  # s-chunk i at [:, 512i:+512]
        for i in range(NT):
            pv = psum.tile([P, 512], f32, tag="ps")
            for c in range(NCC):
                nc.tensor.matmul(
                    pv,
                    xT[:, T * c + P * i : T * c + P * (i + 1)],
                    wv_sb[:, 512 * c : 512 * (c + 1)],
                    start=(c == 0),
                    stop=False,
                )
            nc.tensor.matmul(
                pv, ones1, bvrow, start=False, stop=True
            )
            nc.scalar.copy(vsb[:, 512 * i : 512 * (i + 1)], pv)

        # ---- q/k projections, transposed layout [k, t] (k on partitions) ----
        qT = qk.tile([P, NKC * T], f32r, tag="qT")  # k-chunk kc at [:, T*kc:+T]
        kT = qk.tile([P, NKC * T], f32r, tag="kT")
        for w_sb, bias_col, dst in ((wq_sb, 0, qT), (wk_sb, 4, kT)):
            for kc in range(NKC):
                pjs = [psum.tile([P, 512], f32, tag="ps", name=f"pj{j}") for j in range(NJ)]
                for c in range(NCC):
                    lhsT = w_sb[:, 512 * c + P * kc : 512 * c + P * (kc + 1)]
                    for j in range(NJ):
                        nc.tensor.matmul(
                            pjs[j],
                            lhsT,
                            xT[:, T * c + 512 * j : T * c + 512 * (j + 1)],
                            start=(c == 0),
                            stop=(c == NCC - 1),
                        )
                for j in range(NJ):
                    nc.scalar.activation(
                        dst[:, T * kc + 512 * j : T * kc + 512 * (j + 1)],
                        pjs[j],
                        Ident,
                        bias=bqk[:, bias_col + kc : bias_col + kc + 1],
                        scale=1.0,
                    )

        # ---- scores (transposed) + query-axis softmax, strip by strip ----
        # L'[s, t] = sum_k kT[k, s] * qT[k, t]; strip i = s in [128i, 128i+128)
        pP = big.tile([P, OFFS[NT]], bf16, tag="big")  # exp'd scores (strips)
        for i in range(NT):
            j0 = i // 4
            nt = NJ - j0
            pls = [psum.tile([P, 512], f32, tag="ps", name=f"pl{i}_{jj}") for jj in range(nt)]
            for kc in range(NKC):
                lhsT = kT[:, T * kc + P * i : T * kc + P * (i + 1)]
                for jj in range(nt):
                    j = j0 + jj
                    nc.tensor.matmul(
                        pls[jj],
                        lhsT,
                        qT[:, T * kc + 512 * j : T * kc + 512 * (j + 1)],
                        start=(kc == 0),
                        stop=(kc == NKC - 1),
                    )
            # causal mask on the diagonal tile (additive -1e9 below diagonal)
            m = i % 4
            nc.vector.tensor_tensor(pls[0], pls[0], masks[:, 512 * m : 512 * (m + 1)], ADD)
            # exp(scale * L') -> P' (bf16), with per-tile row sums fused
            parts = stat.tile([P, 4], f32, tag="parts")
            for jj in range(nt):
                nc.scalar.activation(
                    pP[:, OFFS[i] + 512 * jj : OFFS[i] + 512 * (jj + 1)],
                    pls[jj],
                    Exp,
                    bias=0.0,
                    scale=SCALE,
                    accum_out=parts[:, jj : jj + 1],
                )
            r = stat.tile([P, 1], f32, tag="r")
            nc.vector.reduce_sum(r, parts[:, 0:nt], axis=AX)
            rinv = stat.tile([P, 1], f32, tag="rinv")
            nc.vector.reciprocal(rinv, r)
            # fold softmax denominator into v: vn[s, :] = v[s, :] / r[s]
            nc.vector.tensor_scalar_mul(
                vsb[:, 512 * i : 512 * (i + 1)], vsb[:, 512 * i : 512 * (i + 1)], rinv
            )

        # ---- read[t, v] = sum_s P'[s, t] * vn[s, v] ----
        for u in range(NT):
            pr = psum.tile([P, 512], f32, tag="ps")
            for i in range(u + 1):
                nc.tensor.matmul(
                    pr,
                    pP[:, OFFS[i] + P * u - T0[i] : OFFS[i] + P * (u + 1) - T0[i]],
                    vsb[:, 512 * i : 512 * (i + 1)],
                    start=(i == 0),
                    stop=(i == u),
                )
            ot = outp.tile([P, V], f32, tag="ot")
            nc.vector.tensor_copy(ot, pr)
            nc.sync.dma_start(out_ap[P * u : P * (u + 1), :], ot)


_CACHE = {}


def _build():
    if "nc" in _CACHE:
        return _CACHE["nc"]
    import concourse.bass as bass
    import concourse.tile as tile
    from concourse import bacc, mybir

    f32 = mybir.dt.float32
    nc = bacc.Bacc("TRN2", target_bir_lowering=False, debug=False)
    x = nc.dram_tensor("x", [T, C], f32, kind="ExternalInput").ap()
    wq = nc.dram_tensor("wq", [C, K], f32, kind="ExternalInput").ap()
    bq = nc.dram_tensor("bq", [K], f32, kind="ExternalInput").ap()
    wk = nc.dram_tensor("wk", [C, K], f32, kind="ExternalInput").ap()
    bk = nc.dram_tensor("bk", [K], f32, kind="ExternalInput").ap()
    wv = nc.dram_tensor("wv", [C, V], f32, kind="ExternalInput").ap()
    bv = nc.dram_tensor("bv", [V], f32, kind="ExternalInput").ap()
    out = nc.dram_tensor("out", [T, V], f32, kind="ExternalOutput").ap()

    with tile.TileContext(nc) as tc:
        emit(tc, out, x, wq, bq, wk, bk, wv, bv)
    nc.compile()
    _CACHE["nc"] = nc
    return nc


def run_device(x, Wq, bq, Wk, bk, Wv, bv, trace=False):
    """Run the sharded kernel; returns (read [B,T,V], BassKernelResults)."""
    from concourse.bass_utils import run_bass_kernel_spmd

    nc = _build()
    f = np.float32
    base = {
        "wq": np.ascontiguousarray(Wq, f),
        "bq": np.ascontiguousarray(bq, f),
        "wk": np.ascontiguousarray(Wk, f),
        "bk": np.ascontiguousarray(bk, f),
        "wv": np.ascontiguousarray(Wv, f),
        "bv": np.ascontiguousarray(bv, f),
    }
    in_maps = [
        dict(base, x=np.ascontiguousarray(x[b], f)) for b in range(NCORES)
    ]
    res = run_bass_kernel_spmd(
        nc, in_maps, core_ids=list(range(NCORES)), trace=trace
    )
    read = np.stack([res.results[b]["out"] for b in range(NCORES)], axis=0)
    return read, res


def kernel(x, Wq, bq, Wk, bk, Wv, bv):
    x = np.asarray(x, np.float32)
    read, _ = run_device(x, Wq, bq, Wk, bk, Wv, bv, trace=False)
    return np.concatenate((x, read), axis=2)
